# revision 2
# baseline (speedup 1.0000x reference)
"""ComENet-style GNN block on 8 Trainium2 NeuronCores (Bass/Tile SPMD).

Sharding: nodes/edges by graph (contiguous graphs per core, cut to balance
edge counts). Edges assigned to the core owning their TARGET node and sorted
by local target, so scatter stays on-device via one-hot matmuls. Source-node
features are host-gathered per edge (pure input rearrangement); all model
arithmetic runs on device. Weights replicated across cores.

Per-core device program:
  1. fold W2@W1 for both edge-feature MLPs (device matmul, fp16 operands)
  2. x_local = swish(x @ lin_w.T + b)  (H-major, fp16 inputs -> f32 out)
  3. per branch: msgs[e] = (feat[e] @ Wc.T) * swish(x_src[e] @ lin_w.T + b)
     (edge-major, fp16 operand matmuls, fp32 PSUM), scatter = one-hot S
     matmuls over data-driven static message windows -> agg (node-major)
     -> PE-transpose -> H-major
  4. conv + lin1/lin2, lincat + residual, 3 residual lins (H-major, f32r)
  5. GraphNorm via per-graph moment matmuls (exact: var = E[h^2]-2a m^2+a^2 m^2)
  6. final linear -> out^T, host reassembles [N, H]

Edge-side matmul operands are fp16 (halves HBM traffic, enables FWL);
node-side stays f32r (output free dim >= 256 -> full rate, exact residuals).
Scatter windows (kstart array + width) are derived from the actual edge
distribution at kernel() time and baked into the program (NEFF-cached per
shape).
"""

import os

os.environ.setdefault("MYCRO_LOCAL_CACHE", "1")

import numpy as np

# ---- problem sizes (hardcoded per contract) ----
N = 12800
E = 51200
H = 256
F1 = 1568
F2 = 224
NG = 64
NCORES = 8
EPS = 1e-5

# ---- sharding capacities ----
NCAP = 2048          # padded local nodes per core
NNC = NCAP // 128    # 16 node chunks
NGC = 16             # local graph slots per core
KP = 112             # matmul K-chunk rows (F1 = 14*112, F2 = 2*112)
KF1 = 14
KF2 = 2

_PROG_CACHE = {}


# ======================================================================
# Device program
# ======================================================================

def _build_program(nkc, kstart, wstat):
    """nkc: edge chunks; kstart: per-node-chunk window start (len NNC);
    wstat: uniform window width."""
    import concourse.bass as bass
    import concourse.mybir as mybir
    import concourse.tile as tile
    from concourse import bacc
    from concourse.masks import make_identity

    f32 = mybir.dt.float32
    f32r = mybir.dt.float32r
    f16 = mybir.dt.float16
    AF = mybir.ActivationFunctionType
    NKC = nkc
    WSTAT = wstat
    MRING = WSTAT + 3

    nc = bacc.Bacc("TRN2", target_bir_lowering=False, debug=False,
                   num_devices=NCORES)

    def din(name, shape, dt=f32r):
        return nc.dram_tensor(name, shape, dt, kind="ExternalInput")

    QG = 4  # chunks per quad load of f2/xs streams
    NQ = NKC // QG

    # -- data shards (edge side fp16)
    f1t_d = din("f1t", [NKC, KP, KF1 * 128], f16)
    f2t_d = din("f2t", [NQ, KP, QG * KF2 * 128], f16)
    xsrct_d = din("xsrct", [NQ, 128, QG * 2 * 128], f16)
    xloct_d = din("xloct", [H, NCAP], f16)
    s_d = din("s_oh", [NNC, 128, WSTAT, 128], f16)
    g_d = din("g_oh", [NNC, 128, NGC])
    gt_d = din("gt_oh", [NGC, NCAP])
    # -- weights (replicated)
    w1_d = din("w1", [H, F1], f16)
    w2t1_d = din("w2t1", [H, H], f16)
    w12_d = din("w12", [H, F2], f16)
    w2t2_d = din("w2t2", [H, H], f16)
    linwt16_d = din("linwt16", [H, H], f16)
    c1llt_d = din("c1llt", [H, H])
    c1lrt_d = din("c1lrt", [H, H])
    c2llt_d = din("c2llt", [H, H])
    c2lrt_d = din("c2lrt", [H, H])
    lin1t_d = din("lin1t", [H, H])
    lin2t_d = din("lin2t", [H, H])
    lincatt_d = din("lincatt", [2 * H, H])
    linst_d = din("linst", [3 * H, H])
    finalt_d = din("finalt", [H, H])
    linb_row_d = din("linb_row", [1, H])
    linb_pp_d = din("linb_pp", [128, 2], f32)
    c1llb_pp_d = din("c1llb_pp", [128, 2], f32)
    c2llb_pp_d = din("c2llb_pp", [128, 2], f32)
    lin1b_pp_d = din("lin1b_pp", [128, 2], f32)
    lin2b_pp_d = din("lin2b_pp", [128, 2], f32)
    lincatb_pp_d = din("lincatb_pp", [128, 2], f32)
    linsb_pp_d = din("linsb_pp", [128, 6], f32)
    finalb_pp_d = din("finalb_pp", [128, 2], f32)
    gamma_pp_d = din("gamma_pp", [128, 2], f32)
    beta_pp_d = din("beta_pp", [128, 2], f32)
    alpha_row_d = din("alpha_row", [1, H], f32)
    invcnt_d = din("invcnt", [NGC, 1], f32)

    outt_d = nc.dram_tensor("outt", [H, NCAP], f32, kind="ExternalOutput")

    from contextlib import ExitStack

    with tile.TileContext(nc) as tc, ExitStack() as stack:
        const = stack.enter_context(tc.tile_pool(name="const", bufs=1))
        big = stack.enter_context(tc.tile_pool(name="big", bufs=1))
        stream = stack.enter_context(tc.tile_pool(name="stream", bufs=2))
        s3 = stack.enter_context(tc.tile_pool(name="s3", bufs=3))
        spool = stack.enter_context(tc.tile_pool(name="spool", bufs=2))
        psum = stack.enter_context(tc.tile_pool(name="psum", bufs=1, space="PSUM"))
        psumd = stack.enter_context(tc.tile_pool(name="psumd", bufs=2, space="PSUM"))

        def load_w2(d, dt=f32r, pool=None):  # [256, X] -> sbuf [128, 2, X]
            t = (pool or const).tile([128, 2, d.shape[1]], dt, tag=f"w_{d.name}")
            nc.sync.dma_start(out=t[:], in_=d[:].rearrange("(a p) n -> p a n", p=128))
            return t

        def load_pp(d):
            t = const.tile([128, d.shape[1]], f32, tag=f"pp_{d.name}")
            nc.sync.dma_start(out=t[:], in_=d[:])
            return t

        def silu_act(out_ap, in_ap, bias_ap=None):
            if bias_ap is None:
                nc.scalar.activation(out=out_ap, in_=in_ap, func=AF.Silu)
            else:
                nc.scalar.activation(out=out_ap, in_=in_ap, func=AF.Silu,
                                     bias=bias_ap)

        linwt16 = load_w2(linwt16_d, dt=f16)
        c1llt = load_w2(c1llt_d)
        c1lrt = load_w2(c1lrt_d)
        c2llt = load_w2(c2llt_d)
        c2lrt = load_w2(c2lrt_d)
        lin1t = load_w2(lin1t_d)
        lin2t = load_w2(lin2t_d)
        finalt = load_w2(finalt_d)
        lincatt = const.tile([128, 4, H], f32r)
        nc.sync.dma_start(out=lincatt[:], in_=lincatt_d[:].rearrange("(a p) n -> p a n", p=128))
        linst = const.tile([128, 6, H], f32r)
        nc.sync.dma_start(out=linst[:], in_=linst_d[:].rearrange("(a p) n -> p a n", p=128))

        linb_pp = load_pp(linb_pp_d)
        c1llb_pp = load_pp(c1llb_pp_d)
        c2llb_pp = load_pp(c2llb_pp_d)
        lin1b_pp = load_pp(lin1b_pp_d)
        lin2b_pp = load_pp(lin2b_pp_d)
        lincatb_pp = load_pp(lincatb_pp_d)
        linsb_pp = load_pp(linsb_pp_d)
        finalb_pp = load_pp(finalb_pp_d)
        gamma_pp = load_pp(gamma_pp_d)
        beta_pp = load_pp(beta_pp_d)

        linb_bc = const.tile([128, H], f32)
        nc.sync.dma_start(out=linb_bc[:], in_=linb_row_d[:].bitcast(f32).to_broadcast((128, H)))
        alpha16 = const.tile([NGC, H], f32)
        nc.sync.dma_start(out=alpha16[:], in_=alpha_row_d[:].to_broadcast((NGC, H)))
        inv_cnt = const.tile([NGC, 1], f32)
        nc.sync.dma_start(out=inv_cnt[:], in_=invcnt_d[:])

        ident = const.tile([128, 128], f32)
        make_identity(nc, ident[:])

        g_oh = const.tile([128, NNC, NGC], f32r)
        nc.sync.dma_start(out=g_oh[:], in_=g_d[:].rearrange("c p g -> p c g"))
        gt_oh = const.tile([NGC, NCAP], f32r)
        nc.sync.dma_start(out=gt_oh[:], in_=gt_d[:])

        # ---- fold combined edge-MLP weights: WcT = W1T @ W2T (streamed) ----
        wc1t = const.tile([KP, KF1, H], f16)
        wc2t = const.tile([KP, KF2, H], f16)
        w2t1_sb = load_w2(w2t1_d, dt=f16, pool=stream)
        w2t2_sb = load_w2(w2t2_d, dt=f16, pool=stream)
        for wct, wsrc, KF in ((wc1t, w1_d, KF1), (wc2t, w12_d, KF2)):
            for fk in range(KF):
                wtile = stream.tile([128, 2, KP], f16, tag="wfold")
                nc.sync.dma_start(out=wtile[:],
                                  in_=wsrc[:, fk * KP:(fk + 1) * KP].rearrange("(a p) f -> p a f", p=128))
                ps = psum.tile([KP, H], f32, tag="pE")
                rhs = w2t1_sb if wct is wc1t else w2t2_sb
                for hc in range(2):
                    nc.tensor.matmul(ps[:], lhsT=wtile[:, hc, :], rhs=rhs[:, hc, :],
                                     start=(hc == 0), stop=(hc == 1))
                nc.vector.tensor_copy(out=wct[:, fk, :], in_=ps[:])

        # ---- x_local projection (H-major, fused bias+swish) ----
        xlocT = big.tile([128, 2, NCAP], f32r, tag="xlocT")
        for n4 in range(NCAP // 512):
            xlr = s3.tile([128, 2, 512], f16, tag="xlr")
            nc.sync.dma_start(out=xlr[:],
                              in_=xloct_d[:, n4 * 512:(n4 + 1) * 512].rearrange("(a p) n -> p a n", p=128))
            for ho in range(2):
                ps = psum.tile([128, 512], f32, tag="pE")
                for hc in range(2):
                    nc.tensor.matmul(ps[:], lhsT=linwt16[:, hc, ho * 128:(ho + 1) * 128],
                                     rhs=xlr[:, hc, :],
                                     start=(hc == 0), stop=(hc == 1))
                silu_act(xlocT[:, ho, n4 * 512:(n4 + 1) * 512], ps[:],
                         linb_pp[:, ho:ho + 1])

        # ---- merged branches: produce msgs for both, scatter both, eager conv ----
        trigger = {k: [] for k in range(NKC)}
        for c in range(NNC):
            trigger[kstart[c] + WSTAT - 1].append(c)

        msgs1 = big.tile([128, MRING, H], f16, tag="msgs1")
        msgs2 = big.tile([128, MRING, H], f16, tag="msgs2")
        hcat = big.tile([128, 2, NCAP], f32r, tag="hcat")
        hT = big.tile([128, 2, NCAP], f32r, tag="hT")
        agg_cur = [None, None]   # rolling [128, 2, 512] aggT tiles per branch

        def produce_chunk(k):
            # branch-1 features (one DMA, 14 K-chunks of 112)
            ftile = stream.tile([KP, KF1, 128], f16, tag="ftile1")
            nc.sync.dma_start(out=ftile[:],
                              in_=f1t_d[k].rearrange("p (o f) -> p o f", o=KF1))
            ps_f1 = psum.tile([128, H], f32, tag="pA")
            for kc in range(KF1):
                nc.tensor.matmul(ps_f1[:], lhsT=ftile[:, kc, :], rhs=wc1t[:, kc, :],
                                 start=(kc == 0), stop=(kc == KF1 - 1))
            if k % QG == 0:
                f2quad = stream.tile([KP, QG, KF2, 128], f16, tag="f2quad")
                nc.sync.dma_start(out=f2quad[:],
                                  in_=f2t_d[k // QG].rearrange("p (b o f) -> p b o f", b=QG, o=KF2))
                xsquad = stream.tile([128, QG, 2, 128], f16, tag="xsquad")
                nc.sync.dma_start(out=xsquad[:],
                                  in_=xsrct_d[k // QG].rearrange("p (b a e) -> p b a e", b=QG, a=2))
                produce_chunk.f2quad = f2quad
                produce_chunk.xsquad = xsquad
            f2quad, xsquad = produce_chunk.f2quad, produce_chunk.xsquad
            b = k % QG
            ps_f2 = psumd.tile([128, H], f32, tag="pB")
            for kc in range(KF2):
                nc.tensor.matmul(ps_f2[:], lhsT=f2quad[:, b, kc, :], rhs=wc2t[:, kc, :],
                                 start=(kc == 0), stop=(kc == KF2 - 1))
            ps_x = psumd.tile([128, H], f32, tag="pB")
            nc.tensor.matmul(ps_x[:], lhsT=xsquad[:, b, 0, :], rhs=linwt16[:, 0, :],
                             start=True, stop=False)
            nc.tensor.matmul(ps_x[:], lhsT=xsquad[:, b, 1, :], rhs=linwt16[:, 1, :],
                             start=False, stop=True)
            xs = stream.tile([128, H], f32, tag="xs")
            nc.vector.tensor_add(out=xs[:], in0=ps_x[:], in1=linb_bc[:])
            silu_act(xs[:], xs[:])
            nc.vector.tensor_mul(out=msgs1[:, k % MRING, :], in0=ps_f1[:], in1=xs[:])
            nc.vector.tensor_mul(out=msgs2[:, k % MRING, :], in0=ps_f2[:], in1=xs[:])

        def scatter_chunk(c):
            s_sb = spool.tile([128, WSTAT, 128], f16, tag="s_oh")
            nc.sync.dma_start(out=s_sb[:], in_=s_d[c])
            if c % 4 == 0:
                agg_cur[0] = stream.tile([128, 2, 512], f32r, tag="agg1", name="agg1t")
                agg_cur[1] = stream.tile([128, 2, 512], f32r, tag="agg2", name="agg2t")
            for br, (msgs, ptag, atag) in enumerate(
                    ((msgs1, "pC", "pC"), (msgs2, "pD", "pD"))):
                ps_a = psumd.tile([128, H], f32, tag=ptag)
                for w in range(WSTAT):
                    kk = kstart[c] + w
                    nc.tensor.matmul(ps_a[:], lhsT=s_sb[:, w, :],
                                     rhs=msgs[:, kk % MRING, :],
                                     start=(w == 0), stop=(w == WSTAT - 1))
                agg_nm = stream.tile([128, H], f32, tag="aggnm")
                nc.vector.tensor_copy(out=agg_nm[:], in_=ps_a[:])
                for hc in range(2):
                    ps_t = psumd.tile([128, 128], f32, tag=atag)
                    nc.tensor.transpose(ps_t[:], agg_nm[:, hc * 128:(hc + 1) * 128], ident[:])
                    nc.vector.tensor_copy(
                        out=agg_cur[br][:, hc, (c % 4) * 128:(c % 4 + 1) * 128],
                        in_=ps_t[:])

        def conv_group(n4):
            nsl = slice(n4 * 512, (n4 + 1) * 512)
            for br in range(2):
                aggX = agg_cur[br]
                if br == 0:
                    cllt, clrt, clb, lint, linb_b = c1llt, c1lrt, c1llb_pp, lin1t, lin1b_pp
                else:
                    cllt, clrt, clb, lint, linb_b = c2llt, c2lrt, c2llb_pp, lin2t, lin2b_pp
                inner = s3.tile([128, 2, 512], f32r, tag="n2x512")
                for ho in range(2):
                    hsl = slice(ho * 128, (ho + 1) * 128)
                    ps = psum.tile([128, 512], f32, tag="pE")
                    nc.tensor.matmul(ps[:], lhsT=cllt[:, 0, hsl], rhs=aggX[:, 0, :],
                                     start=True, stop=False)
                    nc.tensor.matmul(ps[:], lhsT=cllt[:, 1, hsl], rhs=aggX[:, 1, :],
                                     start=False, stop=False)
                    nc.tensor.matmul(ps[:], lhsT=clrt[:, 0, hsl], rhs=xlocT[:, 0, nsl],
                                     start=False, stop=False)
                    nc.tensor.matmul(ps[:], lhsT=clrt[:, 1, hsl], rhs=xlocT[:, 1, nsl],
                                     start=False, stop=True)
                    nc.scalar.activation(out=inner[:, ho, :], in_=ps[:], func=AF.Identity,
                                         bias=clb[:, ho:ho + 1])
                hb = s3.tile([128, 2, 512], f32r, tag="n2x512")
                for ho in range(2):
                    hsl = slice(ho * 128, (ho + 1) * 128)
                    ps2 = psum.tile([128, 512], f32, tag="pE")
                    for hc in range(2):
                        nc.tensor.matmul(ps2[:], lhsT=lint[:, hc, hsl],
                                         rhs=inner[:, hc, :],
                                         start=(hc == 0), stop=(hc == 1))
                    silu_act(hb[:, ho, :], ps2[:], linb_b[:, ho:ho + 1])
                for ho in range(2):
                    hsl = slice(ho * 128, (ho + 1) * 128)
                    ps3 = psum.tile([128, 512], f32, tag="pE")
                    for hc in range(2):
                        nc.tensor.matmul(ps3[:], lhsT=lincatt[:, br * 2 + hc, hsl],
                                         rhs=hb[:, hc, :],
                                         start=(hc == 0), stop=(hc == 1))
                    if br == 0:
                        nc.vector.tensor_copy(out=hcat[:, ho, nsl], in_=ps3[:])
                    else:
                        tmp = stream.tile([128, 512], f32, tag="tmp512")
                        nc.vector.tensor_add(out=tmp[:], in0=ps3[:], in1=hcat[:, ho, nsl])
                        nc.scalar.activation(out=tmp[:], in_=tmp[:], func=AF.Identity,
                                             bias=lincatb_pp[:, ho:ho + 1])
                        nc.vector.tensor_add(out=hT[:, ho, nsl], in0=tmp[:],
                                             in1=xlocT[:, ho, nsl])

        for k in range(NKC):
            produce_chunk(k)
            for c in trigger[k]:
                scatter_chunk(c)
                if c % 4 == 3:
                    conv_group(c // 4)

        # ---- residual lins (in place on hT; both ho psums read before writes) ----
        for l in range(3):
            for n4 in range(NCAP // 512):
                nsl = slice(n4 * 512, (n4 + 1) * 512)
                pss = []
                for ho in range(2):
                    hsl = slice(ho * 128, (ho + 1) * 128)
                    ps = psumd.tile([128, 512], f32, tag="pB")
                    for hc in range(2):
                        nc.tensor.matmul(ps[:], lhsT=linst[:, l * 2 + hc, hsl],
                                         rhs=hT[:, hc, nsl],
                                         start=(hc == 0), stop=(hc == 1))
                    pss.append(ps)
                for ho in range(2):
                    sw = stream.tile([128, 512], f32, tag="tmp512")
                    silu_act(sw[:], pss[ho][:], linsb_pp[:, l * 2 + ho:l * 2 + ho + 1])
                    nc.vector.tensor_add(out=hT[:, ho, nsl], in0=sw[:], in1=hT[:, ho, nsl])

        # ---- GraphNorm ----
        h_nm = big.tile([128, NNC, H], f32r, tag="xlocT")
        for c in range(NNC):
            for hc in range(2):
                ps_t = psumd.tile([128, 128], f32, tag="pC")
                nc.tensor.transpose(ps_t[:], hT[:, hc, c * 128:(c + 1) * 128].bitcast(f32),
                                    ident[:])
                nc.vector.tensor_copy(out=h_nm[:, c, hc * 128:(hc + 1) * 128], in_=ps_t[:])
        sq_nm = big.tile([128, NNC, H], f32r, tag="hcat")
        nc.vector.tensor_mul(out=sq_nm[:], in0=h_nm[:], in1=h_nm[:])

        ps_sh = psum.tile([NGC, H], f32, tag="pA")
        ps_sq = psumd.tile([NGC, H], f32, tag="pB")
        for c in range(NNC):
            nc.tensor.matmul(ps_sh[:], lhsT=g_oh[:, c, :], rhs=h_nm[:, c, :],
                             start=(c == 0), stop=(c == NNC - 1))
            nc.tensor.matmul(ps_sq[:], lhsT=g_oh[:, c, :], rhs=sq_nm[:, c, :],
                             start=(c == 0), stop=(c == NNC - 1))
        mean = const.tile([NGC, H], f32)
        nc.vector.tensor_tensor(out=mean[:], in0=ps_sh[:],
                                in1=inv_cnt[:].to_broadcast((NGC, H)),
                                op=mybir.AluOpType.mult)
        meansq = const.tile([NGC, H], f32)
        nc.vector.tensor_tensor(out=meansq[:], in0=ps_sq[:],
                                in1=inv_cnt[:].to_broadcast((NGC, H)),
                                op=mybir.AluOpType.mult)
        am = const.tile([NGC, H], f32r)
        nc.vector.tensor_mul(out=am[:], in0=alpha16[:], in1=mean[:])
        t2m = const.tile([NGC, H], f32)
        nc.vector.tensor_scalar_mul(t2m[:], mean[:], 2.0)
        nc.vector.tensor_sub(out=t2m[:], in0=t2m[:], in1=am[:].bitcast(f32))
        nc.vector.tensor_mul(out=t2m[:], in0=am[:].bitcast(f32), in1=t2m[:])
        var = const.tile([NGC, H], f32)
        nc.vector.tensor_sub(out=var[:], in0=meansq[:], in1=t2m[:])
        nc.vector.tensor_scalar_add(var[:], var[:], float(EPS))
        std = const.tile([NGC, H], f32)
        nc.scalar.activation(out=std[:], in_=var[:], func=AF.Sqrt)
        rstd32 = const.tile([NGC, H], f32)
        nc.vector.reciprocal(out=rstd32[:], in_=std[:])
        rstd = const.tile([NGC, H], f32r)
        nc.vector.tensor_copy(out=rstd[:], in_=rstd32[:])

        for n4 in range(NCAP // 512):
            nsl = slice(n4 * 512, (n4 + 1) * 512)
            for ho in range(2):
                hsl = slice(ho * 128, (ho + 1) * 128)
                ps_am = psumd.tile([128, 512], f32, tag="pC")
                nc.tensor.matmul(ps_am[:], lhsT=am[:, hsl], rhs=gt_oh[:, nsl],
                                 start=True, stop=True)
                ps_rs = psumd.tile([128, 512], f32, tag="pD")
                nc.tensor.matmul(ps_rs[:], lhsT=rstd[:, hsl], rhs=gt_oh[:, nsl],
                                 start=True, stop=True)
                t = stream.tile([128, 512], f32, tag="tmp512")
                nc.vector.tensor_sub(out=t[:], in0=hT[:, ho, nsl], in1=ps_am[:])
                nc.vector.tensor_mul(out=t[:], in0=t[:], in1=ps_rs[:])
                nc.scalar.activation(out=hT[:, ho, nsl], in_=t[:], func=AF.Identity,
                                     scale=gamma_pp[:, ho:ho + 1],
                                     bias=beta_pp[:, ho:ho + 1])

        # ---- final linear ----
        outt_r = outt_d[:].rearrange("(a p) n -> p a n", p=128)
        for n4 in range(NCAP // 512):
            nsl = slice(n4 * 512, (n4 + 1) * 512)
            for ho in range(2):
                hsl = slice(ho * 128, (ho + 1) * 128)
                ps = psumd.tile([128, 512], f32, tag="pB")
                for hc in range(2):
                    nc.tensor.matmul(ps[:], lhsT=finalt[:, hc, hsl],
                                     rhs=hT[:, hc, nsl],
                                     start=(hc == 0), stop=(hc == 1))
                ot = stream.tile([128, 512], f32, tag="tmp512")
                nc.scalar.activation(out=ot[:], in_=ps[:], func=AF.Identity,
                                     bias=finalb_pp[:, ho:ho + 1])
                nc.sync.dma_start(out=outt_r[:, ho, nsl], in_=ot[:])

    nc.compile()
    return nc


def _get_program(key=None):
    if key is None:
        key = _PROG_CACHE.get("last")
        assert key is not None, "call _shard first"
    if key not in _PROG_CACHE:
        _PROG_CACHE[key] = _build_program(*key)
    _PROG_CACHE["last"] = key
    return _PROG_CACHE[key]


# ======================================================================
# Host-side sharding
# ======================================================================

def _pp(b):  # [256] -> per-partition [128, 2] (ho-chunk columns)
    return np.ascontiguousarray(b.reshape(2, 128).T, dtype=np.float32)


def _shared_weights(inp):
    f32 = np.float32
    f16 = np.float16
    w = {}
    w["w1"] = np.ascontiguousarray(inp["f1_w1"], f16)
    w["w2t1"] = np.ascontiguousarray(np.asarray(inp["f1_w2"], f32).T.astype(f16))
    w["w12"] = np.ascontiguousarray(inp["f2_w1"], f16)
    w["w2t2"] = np.ascontiguousarray(np.asarray(inp["f2_w2"], f32).T.astype(f16))
    w["linwt16"] = np.ascontiguousarray(np.asarray(inp["lin_w"], f32).T.astype(f16))
    for name, key in [("c1llt", "c1_ll_w"), ("c1lrt", "c1_lr_w"),
                      ("c2llt", "c2_ll_w"), ("c2lrt", "c2_lr_w"),
                      ("lin1t", "lin1_w"), ("lin2t", "lin2_w"), ("finalt", "final_w")]:
        w[name] = np.ascontiguousarray(np.asarray(inp[key], f32).T)
    w["lincatt"] = np.ascontiguousarray(np.asarray(inp["lincat_w"], f32).T)  # [512,256]
    w["linst"] = np.ascontiguousarray(
        np.concatenate([np.asarray(inp["lins_w"][l], f32).T for l in range(3)], axis=0))
    w["linb_row"] = np.asarray(inp["lin_b"], f32).reshape(1, H).copy()
    w["linb_pp"] = _pp(np.asarray(inp["lin_b"], f32))
    w["c1llb_pp"] = _pp(np.asarray(inp["c1_ll_b"], f32))
    w["c2llb_pp"] = _pp(np.asarray(inp["c2_ll_b"], f32))
    w["lin1b_pp"] = _pp(np.asarray(inp["lin1_b"], f32))
    w["lin2b_pp"] = _pp(np.asarray(inp["lin2_b"], f32))
    w["lincatb_pp"] = _pp(np.asarray(inp["lincat_b"], f32))
    w["linsb_pp"] = np.concatenate(
        [_pp(np.asarray(inp["lins_b"][l], f32)) for l in range(3)], axis=1)  # [128, 6]
    w["finalb_pp"] = _pp(np.asarray(inp["final_b"], f32))
    w["gamma_pp"] = _pp(np.asarray(inp["norm_gamma"], f32))
    w["beta_pp"] = _pp(np.asarray(inp["norm_beta"], f32))
    w["alpha_row"] = np.asarray(inp["norm_alpha"], f32).reshape(1, H).copy()
    return w


def _shard(inp):
    f32 = np.float32
    f16 = np.float16
    x = np.asarray(inp["x"], f32)
    f1 = np.asarray(inp["feature1"], f32)
    f2 = np.asarray(inp["feature2"], f32)
    ei = np.asarray(inp["edge_index"]).astype(np.int64)
    batch = np.asarray(inp["batch"]).astype(np.int64)
    src, tgt = ei[0], ei[1]

    gn_counts = np.bincount(batch, minlength=NG)          # nodes per graph
    ge_counts = np.bincount(batch[tgt], minlength=NG)     # edges per graph (by target)
    gn_start = np.concatenate([[0], np.cumsum(gn_counts)])

    # contiguous graph partition balancing edges
    cume = np.cumsum(ge_counts)
    bounds = [0]
    for c in range(1, NCORES):
        target = cume[-1] * c / NCORES
        g = int(np.searchsorted(cume, target))
        bounds.append(max(bounds[-1] + 1, min(g + 1, NG - (NCORES - c))))
    bounds.append(NG)

    # per-core edge sets / local targets (first pass: geometry only)
    cores = []
    max_e = 0
    for c in range(NCORES):
        glo, ghi = bounds[c], bounds[c + 1]
        ns, ne = int(gn_start[glo]), int(gn_start[ghi])
        ncnt = ne - ns
        assert ncnt <= NCAP, f"core {c}: {ncnt} nodes > NCAP"
        assert ghi - glo <= NGC, f"core {c}: {ghi - glo} graphs > NGC"
        emask = (tgt >= ns) & (tgt < ne)
        eidx = np.nonzero(emask)[0]
        loc_t = tgt[eidx] - ns
        order = np.argsort(loc_t, kind="stable")
        eidx = eidx[order]
        loc_t = loc_t[order]
        cores.append((glo, ghi, ns, ne, eidx, loc_t))
        max_e = max(max_e, len(eidx))

    # edge capacity: multiple of 4*128 for quad loads
    nkc = -(-max_e // 128)
    nkc = -(-nkc // 4) * 4
    ecap = nkc * 128

    # data-driven static scatter windows: kstart[c] = min over cores of the
    # first edge chunk feeding node chunk c; wstat covers the max span.
    start_c = np.full(NNC, nkc, dtype=np.int64)
    end_c = np.zeros(NNC, dtype=np.int64)
    for (_, _, _, _, eidx, loc_t) in cores:
        cum = np.searchsorted(loc_t, np.arange(NNC + 1) * 128)
        for c in range(NNC):
            lo, hi = cum[c], cum[c + 1]
            ks = lo // 128 if hi > lo else min(lo // 128, nkc - 1)
            ke = (hi - 1) // 128 if hi > lo else ks
            start_c[c] = min(start_c[c], ks)
            end_c[c] = max(end_c[c], ke)
    # enforce monotone starts (searchsorted cums are monotone per core; the
    # min over cores stays monotone, but clamp defensively)
    for c in range(1, NNC):
        start_c[c] = max(start_c[c], start_c[c - 1])
    wstat = int((end_c - start_c).max()) + 1
    kstart = [int(min(start_c[c], nkc - wstat)) for c in range(NNC)]

    w = _shared_weights(inp)
    in_maps = []
    meta = []
    for c in range(NCORES):
        glo, ghi, ns, ne, eidx, loc_t = cores[c]
        ncnt = ne - ns
        ecnt = len(eidx)

        f1_sh = np.zeros((ecap, F1), f16)
        f1_sh[:ecnt] = f1[eidx]
        # [NKC, KP, KF1*128]: partition p holds K-chunk rows (o*KP+p) contiguous
        f1t = np.ascontiguousarray(
            f1_sh.reshape(nkc, 128, KF1, KP).transpose(0, 3, 2, 1).reshape(nkc, KP, KF1 * 128))
        f2_sh = np.zeros((ecap, F2), f16)
        f2_sh[:ecnt] = f2[eidx]
        # quad layout [NKC/4, KP, 4*KF2*128]
        f2t = np.ascontiguousarray(
            f2_sh.reshape(nkc // 4, 4, 128, KF2, KP).transpose(0, 4, 1, 3, 2)
            .reshape(nkc // 4, KP, 4 * KF2 * 128))
        xs_sh = np.zeros((ecap, H), f16)
        xs_sh[:ecnt] = x[src[eidx]]
        xsrct = np.ascontiguousarray(
            xs_sh.reshape(nkc // 4, 4, 128, 2, 128).transpose(0, 4, 1, 3, 2)
            .reshape(nkc // 4, 128, 4 * 2 * 128))
        xloc = np.zeros((NCAP, H), f32)
        xloc[:ncnt] = x[ns:ne]
        xloct = np.ascontiguousarray(xloc.T.astype(f16))

        slots = np.arange(ecnt)
        kk = slots // 128
        cc = loc_t // 128
        ww = kk - np.asarray(kstart)[cc]
        assert (ww >= 0).all() and (ww < wstat).all(), f"core {c}: window overflow"
        s_oh = np.zeros((NNC, 128, wstat, 128), f16)
        s_oh[cc, slots % 128, ww, loc_t - cc * 128] = 1

        g_loc = batch[ns:ne] - glo
        nl = np.arange(ncnt)
        g_oh = np.zeros((NNC, 128, NGC), f32)
        g_oh[nl // 128, nl % 128, g_loc] = 1
        gt_oh = np.zeros((NGC, NCAP), f32)
        gt_oh[g_loc, nl] = 1
        cnt = np.bincount(g_loc, minlength=NGC).astype(f32)
        invcnt = (1.0 / np.maximum(cnt, 1.0)).reshape(NGC, 1).astype(f32)

        m = {"f1t": f1t, "f2t": f2t, "xsrct": xsrct, "xloct": xloct,
             "s_oh": s_oh, "g_oh": g_oh, "gt_oh": gt_oh, "invcnt": invcnt}
        m.update(w)
        in_maps.append(m)
        meta.append((ns, ne))
    return in_maps, meta, (nkc, tuple(kstart), wstat)


def kernel(**inputs):
    from concourse.bass_utils import run_bass_kernel_spmd

    in_maps, meta, key = _shard(inputs)
    nc = _get_program(key)
    res = run_bass_kernel_spmd(nc, in_maps, list(range(NCORES)))
    out = np.empty((N, H), np.float32)
    for c, (ns, ne) in enumerate(meta):
        out[ns:ne] = res.results[c]["outt"][:, :ne - ns].T
    return out
